# revision 1
# baseline (speedup 1.0000x reference)
"""MinLSTM Trainium2 kernel: B=8, S=8192, D=512, H=256, 8 NeuronCores.

Strategy: data-parallel over batch (one sequence per core). Per core:
  yT[3H, S] = Wt.T-chunks @ xT-chunks via PE (float32r, full-rate fp32),
  gates from PSUM on ACT/DVE, linear-space recurrence
  h_t = F*h_{t-1} + (1-F)*G via the DVE tensor_tensor_scan instruction.

The reference's log-space cumlogsumexp scan is mathematically the linear
recurrence h_t = f_t*h_{t-1} + i_t*g(h~_t) with normalized gates
F = sigmoid(-(softplus(-f)-softplus(-i))), I = 1-F, and
g(z) = max(sigmoid(z), z+0.5) (the two branches cross at z=0).
The linear recurrence is contraction-stable (F<1), so fp32 suffices.

Host-side staging (not on the HW critical path): transpose x to [D, S]
per batch so the PE's contraction dim (D) lands on SBUF partitions with
S contiguous, transpose W to [D, 3H], and fold g() into the initial
carry h0 = g(h_prev). Output is produced as [H, S] per core and
transposed back on the host.
"""

import sys

import numpy as np

sys.path.insert(0, "/opt/trn_rl_repo")

B, S, D, H = 8, 8192, 512, 256
S_TILE = 512
N_TILES = S // S_TILE
K_CH = D // 128  # 4 contraction chunks
N_CORES = 8

_cache = {}


BF16_SCAN = False  # bf16 F/mV/out: ~2x faster DVE scan path, ~0.3% extra error


def _build_nc(act_subst=None):
    from contextlib import ExitStack

    import concourse.bacc as bacc
    import concourse.tile as tile
    from concourse import mybir

    f32 = mybir.dt.float32
    f32r = mybir.dt.float32r
    Alu = mybir.AluOpType
    Act = mybir.ActivationFunctionType

    bf16 = mybir.dt.bfloat16
    sdt = bf16 if BF16_SCAN else f32

    nc = bacc.Bacc("TRN2", target_bir_lowering=False)
    xt = nc.dram_tensor("xt", [D, S], f32r, kind="ExternalInput")
    wt = nc.dram_tensor("wt", [D, 3 * H], f32r, kind="ExternalInput")
    h0 = nc.dram_tensor("h0", [H, 1], f32, kind="ExternalInput")
    out = nc.dram_tensor("out", [H, S], sdt, kind="ExternalOutput")

    with tile.TileContext(nc) as tc, ExitStack() as ctx:
        const_pool = ctx.enter_context(tc.tile_pool(name="const", bufs=1))
        xin_pool = ctx.enter_context(tc.tile_pool(name="xin", bufs=4))
        fi_pool = ctx.enter_context(tc.tile_pool(name="fi_ps", bufs=2, space="PSUM"))
        h_pool = ctx.enter_context(tc.tile_pool(name="h_ps", bufs=2, space="PSUM"))
        work = ctx.enter_context(tc.tile_pool(name="work", bufs=3))
        hout_pool = ctx.enter_context(tc.tile_pool(name="hout", bufs=4))

        wt_view = wt.rearrange("(k p) n -> p k n", p=128)
        wt_sb = []
        for k in range(K_CH):
            wtk = const_pool.tile([128, 3 * H], f32r, name=f"wt{k}", tag=f"wt{k}")
            nc.gpsimd.dma_start(out=wtk, in_=wt_view[:, k, :])
            wt_sb.append(wtk)
        # carries hold u = -h (negated state; undone on the host)
        h0_sb = const_pool.tile([128, 2], f32)
        nc.gpsimd.dma_start(out=h0_sb, in_=h0.rearrange("(c p) one -> p (c one)", p=128))
        carry = [h0_sb[:, 0:1], h0_sb[:, 1:2]]

        xt_view = xt.rearrange("(k p) s -> p k s", p=128)

        for t in range(N_TILES):
            sl = slice(t * S_TILE, (t + 1) * S_TILE)
            xt_sb = []
            for k in range(K_CH):
                xtk = xin_pool.tile([128, S_TILE], f32r, name=f"xt{k}", tag=f"xt{k}")
                nc.sync.dma_start(out=xtk, in_=xt_view[:, k, sl])
                xt_sb.append(xtk)

            # Steady state batches gate math across both H-chunks (FD=1024,
            # lower per-instruction overhead). Tile 0 runs it per-phase so
            # the DVE/scan chain starts as soon as phase 0's gates exist.
            batched = t > 0

            # ss = [sf0 | si0 | sf1 | si1], sh/gg = [c0 | c1] along free dim
            ss = work.tile([128, 4 * S_TILE], f32, tag="ss")
            gg = work.tile([128, 2 * S_TILE], f32, tag="gg")
            ssv = ss.rearrange("p (c x s) -> p c x s", c=2, x=2)
            s2 = work.tile([128, 2, S_TILE], f32, tag="s2")
            rr = work.tile([128, 2, S_TILE], f32, tag="rr")
            ff = work.tile([128, 2, S_TILE], sdt, tag="ff")
            mv = work.tile([128, 2 * S_TILE], sdt, tag="mv")
            h_ps = h_pool.tile([128, 2 * S_TILE], f32)
            for c in range(2):
                fi_ps = fi_pool.tile([128, 2 * S_TILE], f32)
                csl = slice(c * S_TILE, (c + 1) * S_TILE)
                for k in range(K_CH):
                    rhs = xt_sb[k]
                    st = dict(start=(k == 0), stop=(k == K_CH - 1))
                    nc.tensor.matmul(
                        fi_ps[:, 0:S_TILE],
                        lhsT=wt_sb[k][:, c * 128 : c * 128 + 128],
                        rhs=rhs, **st)
                    nc.tensor.matmul(
                        fi_ps[:, S_TILE : 2 * S_TILE],
                        lhsT=wt_sb[k][:, H + c * 128 : H + c * 128 + 128],
                        rhs=rhs, **st)
                    nc.tensor.matmul(
                        h_ps[:, csl],
                        lhsT=wt_sb[k][:, 2 * H + c * 128 : 2 * H + c * 128 + 128],
                        rhs=rhs, **st)

                # [sigmoid(f) | sigmoid(i)] in one ACT pass over both banks
                nc.scalar.activation(ss[:, 2 * c * S_TILE : 2 * (c + 1) * S_TILE],
                                     fi_ps, Act.Sigmoid)
                if not batched:
                    sh = work.tile([128, S_TILE], f32, tag="sh")
                    nc.scalar.activation(sh, h_ps[:, csl], Act.Sigmoid)
                    nc.vector.scalar_tensor_tensor(
                        gg[:, csl], in0=h_ps[:, csl], scalar=0.5, in1=sh,
                        op0=Alu.add, op1=Alu.max)
                    nc.vector.tensor_add(s2[:, c, :], ssv[:, c, 0, :], ssv[:, c, 1, :])
                    nc.vector.reciprocal_approx_fast(out=rr[:, c, :], in_=s2[:, c, :])
                    nc.vector.tensor_mul(ff[:, c, :], ssv[:, c, 0, :], rr[:, c, :])
                    nc.vector.scalar_tensor_tensor(
                        mv[:, csl], in0=ff[:, c, :], scalar=-1.0, in1=gg[:, csl],
                        op0=Alu.add, op1=Alu.mult)

            if batched:
                # G = max(sigmoid(h), h+0.5) == sigmoid(-relu(-h)) + relu(h)
                # (exact: h>=0 -> 0.5+h, h<0 -> sigmoid(h)). Built without
                # touching the DVE: three ACT passes + an add-reduce SWDGE DMA.
                rn = work.tile([128, 2 * S_TILE], f32, tag="rn")
                nc.scalar.activation(rn, h_ps, Act.Relu, scale=-1.0)
                nc.scalar.activation(gg, rn, Act.Sigmoid, scale=-1.0)
                hc = work.tile([128, 2 * S_TILE], f32, tag="hc")
                nc.scalar.activation(hc, h_ps, Act.Relu)
                nc.gpsimd.dma_start(out=gg, in_=hc, accum_op=Alu.add)
                # F = sf/(sf+si), mV = (F-1)*G at FD=1024
                sf = ssv[:, :, 0, :]   # [128, 2, S_TILE]
                si = ssv[:, :, 1, :]
                nc.vector.tensor_add(s2, sf, si)
                nc.vector.reciprocal_approx_fast(out=rr, in_=s2)
                nc.vector.tensor_mul(ff, sf, rr)
                nc.vector.scalar_tensor_tensor(
                    mv, in0=ff.rearrange("p c s -> p (c s)"), scalar=-1.0, in1=gg,
                    op0=Alu.add, op1=Alu.mult)
            # scan runs on u = -h: u_t = F*u_{t-1} + mV_t (negation undone on host)
            for c in range(2):
                csl = slice(c * S_TILE, (c + 1) * S_TILE)
                ho = hout_pool.tile([128, S_TILE], sdt, tag=f"ho{c}")
                nc.vector.tensor_tensor_scan(
                    ho, data0=ff[:, c, :], data1=mv[:, csl], initial=carry[c],
                    op0=Alu.mult, op1=Alu.add)
                carry[c] = ho[:, S_TILE - 1 : S_TILE]
                nc.sync.dma_start(out=out[c * 128 : (c + 1) * 128, sl], in_=ho)

    nc.compile()
    return nc


def get_nc():
    if "nc" not in _cache:
        _cache["nc"] = _build_nc()
    return _cache["nc"]


def _stage_inputs(x, h_prev, W):
    """Host-side sharding/layout prep (not on the HW critical path)."""
    x = np.ascontiguousarray(x, dtype=np.float32)
    W = np.ascontiguousarray(W, dtype=np.float32)
    h_prev = np.ascontiguousarray(h_prev, dtype=np.float32)

    wt = np.ascontiguousarray(W.T)  # [D, 3H]
    # carry is u = -h, so feed -g(h_prev); g(z) = z + 0.5 if z >= 0 else sigmoid(z)
    h0 = -np.where(h_prev >= 0, h_prev + 0.5, 1.0 / (1.0 + np.exp(-h_prev)))
    h0 = h0.astype(np.float32)

    in_maps = []
    for b in range(N_CORES):
        in_maps.append({
            "xt": np.ascontiguousarray(x[b].T),       # [D, S]
            "wt": wt,
            "h0": np.ascontiguousarray(h0[b].reshape(H, 1)),
        })
    return in_maps


def kernel(x, h_prev, W):
    from concourse.bass_utils import run_bass_kernel_spmd

    nc = get_nc()
    in_maps = _stage_inputs(x, h_prev, W)
    res = run_bass_kernel_spmd(nc, in_maps, core_ids=list(range(N_CORES)))
    out = np.empty((B, S, H), dtype=np.float32)
    for b in range(N_CORES):
        # kernel scans u = -h; negate while transposing [H, S] -> [S, H]
        u = np.asarray(res.results[b]["out"], dtype=np.float32)
        np.negative(u.T, out=out[b])
    return out


if __name__ == "__main__":
    rng = np.random.default_rng(0)
    x = rng.standard_normal((B, S, D), dtype=np.float32)
    h_prev = rng.standard_normal((B, H), dtype=np.float32)
    W = (rng.standard_normal((3 * H, D), dtype=np.float32) / np.sqrt(D)).astype(np.float32)
    out = kernel(x, h_prev, W)
    print(out.shape, out.dtype, np.abs(out).mean())



# revision 2
# speedup vs baseline: 1.0243x; 1.0243x over previous
"""MinLSTM Trainium2 kernel: B=8, S=8192, D=512, H=256, 8 NeuronCores.

Strategy: data-parallel over batch (one sequence per core). Per core:
  y[3H, S] = W @ x via PE in fp16 (1 cycle/row vs fp32r's ~1.85 — the
  fp32r baseline was PE-bound at 157us busy), gates from PSUM with ONE
  ACT sigmoid pass over a contiguous [f|i|h] 3-bank PSUM tile, fp16
  gate algebra on the DVE with a custom 8-stage fused reciprocal
  (quadratic seed + 1 Newton-Raphson, ~3e-6 rel err), then the linear
  recurrence h_t = F*h_{t-1} + (1-F)*G via tensor_tensor_scan.

The reference's log-space cumlogsumexp scan is mathematically the linear
recurrence h_t = f_t*h_{t-1} + i_t*g(h~_t) with normalized gates
F = sf/(sf+si), I = 1-F (sf=sigmoid(f), si=sigmoid(i)) and
g(z) = max(sigmoid(z), z+0.5) (the two branches cross at z=0).
The scan runs with op1=subtract on M=(F-1)*G: state = F*state - M
= F*state + (1-F)*G, so h comes out directly (no negation).

Numerics (validated in numpy at full size): fp16 GEMM + fp16 gates +
fp16 scan output -> max rel err ~5.6e-3 (gate is 2e-2). bf16 fails
(~2.7e-2); fp16 runs at the same PE rate with 8x finer mantissa.

Host-side staging (not on the HW critical path): transpose x to [D, S]
per batch (contraction dim D on SBUF partitions), cast x/W to fp16,
fold g() into the initial carry h0 = g(h_prev). Output is produced as
[H, S] fp16 per core and transposed back on the host.
"""

import sys

import numpy as np

sys.path.insert(0, "/opt/trn_rl_repo")

B, S, D, H = 8, 8192, 512, 256
S_TILE = 512
N_TILES = S // S_TILE
K_CH = D // 128  # 4 contraction chunks
N_CORES = 8

_cache = {}

# Chebyshev-fit constants for the quadratic-seed reciprocal:
# v = s * bitcast(~s) lands in [-4.5, -4]; p = C0*v + C1 makes
# s*(~s*p) ~ 1 to +-0.17%; one NR pass y*(2 - s*y) finishes at ~3e-6.
_RQNR = dict(s0=-0.05545928135617832, s1=-0.4714038518599473, imm2=2.0)


def _recip_qnr_ref(in0, in1, c0, c1, c2):
    s = in0.astype(np.float32)
    nx = (~s.view(np.int32)).view(np.float32)
    v = (s * nx).astype(np.float32)
    p = (v * np.float32(c0) + np.float32(c1)).astype(np.float32)
    y0 = (nx * p).astype(np.float32)
    return (y0 * (np.float32(c2) - s * y0)).astype(np.float32)


def _ensure_recip_op():
    """Register the fused quad-seed+NR reciprocal as a custom DVE op
    (the documented extension point: append a DveOp to dve_ops.OPS)."""
    from concourse import dve_ops as dops
    from concourse.dve_spec import AluOp, Bin, Spec, Src0, C0, C1, C2, lower
    from concourse.dve_uop import DveOpSpec

    name = "RECIP_QUAD_NR_ANT"
    for op in dops.OPS:
        if op.name == name:
            return op

    nx = Bin(AluOp.BITWISE_NOT, Src0, Src0)
    v = Src0 * nx
    p = v * C0 + C1
    y0 = nx * p
    body = y0 * (C2 - Src0 * y0)
    spec = Spec(body=body, reference=_recip_qnr_ref)

    row = dops._CUSTOM_DVE_ROW_BASE + len(dops.OPS)
    assert row < 0x20
    shas = {}
    for ver in ("v3", "v4"):
        ds = DveOpSpec(name=name, opcode=row, uops=lower(spec, ver=ver))
        shas[ver] = ds.sha(ver)
    op = dops.DveOp(name, spec, subdim=False, uops_sha=shas)
    dops.OPS.append(op)
    dops.CUSTOM_DVE_SPECS[name] = spec
    dops._SUB_OPCODE_FOR_NAME[name] = row
    return op


def _build_nc():
    from contextlib import ExitStack

    import concourse.bacc as bacc
    import concourse.tile as tile
    from concourse import mybir

    f32 = mybir.dt.float32
    f16 = mybir.dt.float16
    Alu = mybir.AluOpType
    Act = mybir.ActivationFunctionType

    recip_op = _ensure_recip_op()

    nc = bacc.Bacc("TRN2", target_bir_lowering=False)
    xt = nc.dram_tensor("xt", [D, S], f16, kind="ExternalInput")
    wt = nc.dram_tensor("wt", [D, 3 * H], f16, kind="ExternalInput")
    h0 = nc.dram_tensor("h0", [H, 1], f32, kind="ExternalInput")
    out = nc.dram_tensor("out", [H, S], f16, kind="ExternalOutput")

    with tile.TileContext(nc) as tc, ExitStack() as ctx:
        const_pool = ctx.enter_context(tc.tile_pool(name="const", bufs=1))
        xin_pool = ctx.enter_context(tc.tile_pool(name="xin", bufs=3))
        ps_pool = ctx.enter_context(tc.tile_pool(name="ps", bufs=2, space="PSUM"))
        sfi_pool = ctx.enter_context(tc.tile_pool(name="sfi", bufs=2))
        work = ctx.enter_context(tc.tile_pool(name="work", bufs=2))
        hout_pool = ctx.enter_context(tc.tile_pool(name="hout", bufs=4))

        wt_view = wt.rearrange("(k p) n -> p k n", p=128)
        wt_sb = []
        for k in range(K_CH):
            wtk = const_pool.tile([128, 3 * H], f16, name=f"wt{k}", tag=f"wt{k}")
            nc.gpsimd.dma_start(out=wtk, in_=wt_view[:, k, :])
            wt_sb.append(wtk)
        h0_sb = const_pool.tile([128, 2], f32)
        nc.gpsimd.dma_start(out=h0_sb, in_=h0.rearrange("(c p) one -> p (c one)", p=128))
        carry = [h0_sb[:, 0:1], h0_sb[:, 1:2]]

        xt_view = xt.rearrange("(k p) s -> p k s", p=128)

        for t in range(N_TILES):
            sl = slice(t * S_TILE, (t + 1) * S_TILE)
            xt_sb = []
            for k in range(K_CH):
                xtk = xin_pool.tile([128, S_TILE], f16, name=f"xt{k}", tag=f"xt{k}")
                nc.sync.dma_start(out=xtk, in_=xt_view[:, k, sl])
                xt_sb.append(xtk)

            # per chunk: [f | i | h] in one 3-bank PSUM tile, one sigmoid
            # pass over all 1536, G right after (releases the PSUM tile)
            sfi = sfi_pool.tile([128, 2, 3 * S_TILE], f16, tag="sfi")
            gg = work.tile([128, 2, S_TILE], f16, tag="gg")
            for c in range(2):
                ps_t = ps_pool.tile([128, 3 * S_TILE], f32)
                for k in range(K_CH):
                    rhs = xt_sb[k]
                    st = dict(start=(k == 0), stop=(k == K_CH - 1))
                    for g in range(3):
                        nc.tensor.matmul(
                            ps_t[:, g * S_TILE : (g + 1) * S_TILE],
                            lhsT=wt_sb[k][:, g * H + c * 128 : g * H + c * 128 + 128],
                            rhs=rhs, **st)
                nc.scalar.activation(sfi[:, c, :], ps_t, Act.Sigmoid)
                # G = max(h + 0.5, sigmoid(h))
                nc.vector.scalar_tensor_tensor(
                    gg[:, c, :], in0=ps_t[:, 2 * S_TILE : 3 * S_TILE], scalar=0.5,
                    in1=sfi[:, c, 2 * S_TILE : 3 * S_TILE], op0=Alu.add, op1=Alu.max)

            sfv = sfi[:, :, 0:S_TILE]
            siv = sfi[:, :, S_TILE : 2 * S_TILE]
            ss = work.tile([128, 2, S_TILE], f16, tag="ss")
            rr = work.tile([128, 2, S_TILE], f16, tag="rr")
            ff = work.tile([128, 2, S_TILE], f16, tag="ff")
            mv = work.tile([128, 2, S_TILE], f16, tag="mv")
            nc.vector.tensor_add(ss, sfv, siv)
            nc.vector._custom_dve(recip_op, out=rr, in0=ss, **_RQNR)
            nc.vector.tensor_mul(ff, sfv, rr)
            nc.vector.scalar_tensor_tensor(
                mv, in0=ff, scalar=-1.0, in1=gg, op0=Alu.add, op1=Alu.mult)

            # h_t = F*h_{t-1} - M = F*h_{t-1} + (1-F)*G
            for c in range(2):
                ho = hout_pool.tile([128, S_TILE], f16, tag=f"ho{c}")
                nc.vector.tensor_tensor_scan(
                    ho, data0=ff[:, c, :], data1=mv[:, c, :], initial=carry[c],
                    op0=Alu.mult, op1=Alu.subtract)
                carry[c] = ho[:, S_TILE - 1 : S_TILE]
                nc.gpsimd.dma_start(out=out[c * 128 : (c + 1) * 128, sl], in_=ho)

    nc.compile()
    return nc


def get_nc():
    if "nc" not in _cache:
        _cache["nc"] = _build_nc()
    return _cache["nc"]


def _stage_inputs(x, h_prev, W):
    """Host-side sharding/layout prep (not on the HW critical path)."""
    x = np.ascontiguousarray(x, dtype=np.float32)
    W = np.ascontiguousarray(W, dtype=np.float32)
    h_prev = np.ascontiguousarray(h_prev, dtype=np.float32)

    wt = np.ascontiguousarray(W.T.astype(np.float16))  # [D, 3H]
    # initial carry h0 = g(h_prev); g(z) = z + 0.5 if z >= 0 else sigmoid(z)
    h0 = np.where(h_prev >= 0, h_prev + 0.5, 1.0 / (1.0 + np.exp(-h_prev)))
    h0 = h0.astype(np.float32)

    in_maps = []
    for b in range(N_CORES):
        in_maps.append({
            "xt": np.ascontiguousarray(x[b].T.astype(np.float16)),  # [D, S]
            "wt": wt,
            "h0": np.ascontiguousarray(h0[b].reshape(H, 1)),
        })
    return in_maps


def kernel(x, h_prev, W):
    from concourse.bass_utils import run_bass_kernel_spmd

    nc = get_nc()
    in_maps = _stage_inputs(x, h_prev, W)
    res = run_bass_kernel_spmd(nc, in_maps, core_ids=list(range(N_CORES)))
    out = np.empty((B, S, H), dtype=np.float32)
    for b in range(N_CORES):
        out[b] = np.asarray(res.results[b]["out"]).T.astype(np.float32)
    return out


if __name__ == "__main__":
    rng = np.random.default_rng(0)
    x = rng.standard_normal((B, S, D), dtype=np.float32)
    h_prev = rng.standard_normal((B, H), dtype=np.float32)
    W = (rng.standard_normal((3 * H, D), dtype=np.float32) / np.sqrt(D)).astype(np.float32)
    out = kernel(x, h_prev, W)
    print(out.shape, out.dtype, np.abs(out).mean())
